# revision 38
# baseline (speedup 1.0000x reference)
"""Multi-head sparse attention on 8 NeuronCores (Trainium2, Bass/Tile).

Head-parallel sharding: core h owns head h (H == n_cores == 8).
Each core computes its head's attention output and the partial final
projection through its W_O column slice; the host sums the 8 partials.

Math note: softmax rows are never fully masked (random 0/1 mask) and
E = QK^T/8 is small, so softmax is computed WITHOUT max subtraction:
P = exp(E) * mask, out = (P @ V) / rowsum(P).  rowsum comes from a
ones-column appended to V, and the normalization is deferred until
after the W_O projection (a per-row scalar).

v7: measured-HW layout.  PE matmul cost is out_free x 1 cycle
regardless of contraction depth or DoubleRow, so E runs as a plain
bf16 matmul with contraction 64 (no fp8 hi/lo folding, no plane-dup
DMAs) -- same speed as the fp8 fold, better precision, far less
phase-1 work.  The mask rides the DVE as a bf16 multiply (2x mode);
the Act engine (exp is only ~640ns/chunk on HW) also absorbs the QK
PSUM drains and the 32 output scalings.  Scheduling: LAG=5 so the
in-order PE never head-of-line blocks on acc drains, V matmuls
interleave into the QK tile loop, xt loads are column-blocked so the
first QK tile lands early, and the last rowgroup's epilogue is split
across DVE/Act/both DMA queues (nothing overlaps the tail).
"""

import numpy as np
import ml_dtypes

H, N, F_IN, HD, F_OUT = 8, 4096, 512, 64, 512
N_CORES = 8
RG = 1024            # query-row group processed per PSUM accumulator
N_RG = N // RG       # 4
MC = 128             # key/m chunk (partition dim)
N_MC = N // MC       # 32
NSPLIT = 512         # matmul moving-operand free size
MB = 4               # mask chunks per DMA batch
NB = N_MC // MB      # mask batches per rowgroup (8)
BF16 = ml_dtypes.bfloat16

_PROGRAM_CACHE = {}


def _build_program(repeat=1, timing=False, variant="full"):
    """Build + compile the Bass/Tile program (same SPMD program for all cores).

    timing=True builds a benchmark variant: inputs live in internal DRAM
    (initialized on-device), the body runs `repeat` times inside a hardware
    For_i loop, and only a tiny checksum output is external.  Differencing
    the wall time of two repeat counts isolates the per-iteration HW time.
    """
    key = (repeat, timing, variant)
    if key in _PROGRAM_CACHE:
        return _PROGRAM_CACHE[key]

    import concourse.bacc as bacc
    import concourse.tile as tile
    import concourse.mybir as mybir

    f32 = mybir.dt.float32
    bf16 = mybir.dt.bfloat16

    nc = bacc.Bacc("TRN2", target_bir_lowering=False, debug=False,
                   num_devices=N_CORES)

    kind_in = {} if timing else {"kind": "ExternalInput"}
    XT = nc.dram_tensor("xt", [F_IN, N], bf16, **kind_in).ap()
    MT = nc.dram_tensor("mt", [N_RG, NB, MB * 128, RG], bf16,
                        **kind_in).ap()
    WQK = nc.dram_tensor("wqk", [128, 4, 128], bf16, **kind_in).ap()
    WV = nc.dram_tensor("wv", [128, 4, HD], bf16, **kind_in).ap()
    WO = nc.dram_tensor("wo", [HD, F_OUT], bf16, **kind_in).ap()
    if not timing:
        OUT = nc.dram_tensor("out", [N, F_OUT], bf16,
                             kind="ExternalOutput").ap()
    else:
        OUT = nc.dram_tensor("out", [N, F_OUT], bf16).ap()
        DUMMY = nc.dram_tensor("dumin", [1, 8], f32, kind="ExternalInput").ap()
        CHK = nc.dram_tensor("chk", [128, F_OUT], bf16,
                             kind="ExternalOutput").ap()

    SCALE = float(1.0 / np.sqrt(HD))

    with tile.TileContext(nc) as tc:
        with (
            tc.tile_pool(name="consts", bufs=1) as consts,
            tc.tile_pool(name="wpool", bufs=1) as wpool,
        ):
            ident11 = consts.tile([1, 1], f32)
            nc.vector.memset(ident11[:], 1.0)
            zeros_p = consts.tile([128, 1], f32)
            nc.vector.memset(zeros_p[:], 0.0)

            wqk_sb = wpool.tile([128, 4, 128], bf16)
            wv_sb = wpool.tile([128, 4, HD], bf16)
            wo_sb = wpool.tile([HD, F_OUT], bf16)
            # fp8 hi/lo folded operands for the DoubleRow E matmul:
            # kfall: stationary planes [128, j, N]; rows 0:64 = (Khi, 0),
            #        rows 64:128 = (Khi copy, Klo)
            # qf2:   moving planes; rows 0:64 = Qlo, rows 64:128 = Qhi,
            #        plane j1 = copy of j0
            fp8 = mybir.dt.float8e4
            kfall = wpool.tile([128, 2, N], fp8)
            qf2 = wpool.tile([128, 2, N], fp8)
            fscr = wpool.tile([128, N], fp8)
            nc.vector.memset(kfall[0:64, 1, :], 0.0)

            if timing:
                # on-device init of internal DRAM inputs (runs once)
                with tc.tile_pool(name="init", bufs=1) as initp:
                    mrow = initp.tile([128, N], bf16)
                    nc.vector.memset(mrow[:], 1.0)
                    MTf = MT.rearrange("g b p q -> (g b p) q")
                    for c in range(N * N // (128 * RG)):
                        nc.sync.dma_start(MTf[c * 128:(c + 1) * 128, :],
                                          mrow[:, 0:RG])
                    xrow = initp.tile([128, N], bf16)
                    nc.vector.memset(xrow[:], 0.015625)
                    for c in range(4):
                        nc.sync.dma_start(XT[c * 128:(c + 1) * 128, :], xrow[:])
                    wrow = initp.tile([128, 4 * 128], bf16)
                    nc.vector.memset(wrow[:], 0.03125)
                    nc.sync.dma_start(WQK.rearrange("p c d -> p (c d)"),
                                      wrow[:])
                    nc.sync.dma_start(WV.rearrange("p c d -> p (c d)"),
                                      wrow[:, 0:4 * HD])
                    worow = initp.tile([HD, F_OUT], bf16)
                    nc.vector.memset(worow[:], 0.03125)
                    nc.sync.dma_start(WO[:], worow[:])

            nc.sync.dma_start(wqk_sb[:], WQK[:])
            nc.sync.dma_start(wv_sb[:], WV[:])
            nc.sync.dma_start(wo_sb[:], WO[:])

            if timing and repeat > 1:
                with tc.For_i(0, repeat, 1):
                    _one_pass(nc, tc, mybir, XT, MT, OUT,
                              wqk_sb, wv_sb, wo_sb, kfall, qf2, fscr,
                              ident11, zeros_p, SCALE, 0, variant)
            else:
                for rep in range(repeat):
                    _one_pass(nc, tc, mybir, XT, MT, OUT,
                              wqk_sb, wv_sb, wo_sb, kfall, qf2, fscr,
                              ident11, zeros_p, SCALE, rep, variant)

            if timing:
                with tc.tile_pool(name="chkp", bufs=1) as chkp:
                    chk_sb = chkp.tile([128, F_OUT], bf16)
                    nc.sync.dma_start(chk_sb[:], OUT[0:128, :])
                    nc.sync.dma_start(CHK[:], chk_sb[:])

    nc.compile()
    _PROGRAM_CACHE[key] = nc
    return nc


def _one_pass(nc, tc, mybir, XT, MT, OUT,
              wqk_sb, wv_sb, wo_sb, kfall, qf2, fscr,
              ident11, zeros_p, SCALE, rep,
              variant="full"):
    f32 = mybir.dt.float32
    bf16 = mybir.dt.bfloat16
    AF = mybir.ActivationFunctionType
    MUL = mybir.AluOpType.mult
    SUB = mybir.AluOpType.subtract
    DR = mybir.MatmulPerfMode.DoubleRow
    r = f"_r{rep}"

    with (
        tc.tile_pool(name="qkv" + r, bufs=1) as qkvpool,
        tc.tile_pool(name="mpool" + r, bufs=4) as mpool,
        tc.tile_pool(name="ppool" + r, bufs=7) as ppool,
        tc.tile_pool(name="fpool" + r, bufs=2) as fpool,
        tc.tile_pool(name="opool" + r, bufs=2) as opool,
    ):
        if variant == "dmaonly":
            dout = opool.tile([128, RG // 128, F_OUT], bf16,
                              name="dout" + r, tag="out")
            nc.vector.memset(dout[:], 0.0)
            for c in range(4):
                xt_c = fpool.tile([128, N], bf16, name=f"dxt_{c}" + r,
                                  tag="tmp")
                eng = nc.sync if (c % 2 == 0) else nc.gpsimd
                eng.dma_start(xt_c[:], XT[c * 128:(c + 1) * 128, :])
            for g in range(N_RG):
                for b in range(NB):
                    mt4 = mpool.tile([128, MB, RG], bf16, name="mt4" + r,
                                     tag="mt")
                    eng = nc.sync if (b % 2 == 0) else nc.gpsimd
                    eng.dma_start(
                        mt4[:],
                        MT[g, b, :, :].rearrange("(k p) q -> p k q", k=MB))
                nc.gpsimd.dma_start(
                    OUT[g * RG:(g + 1) * RG, :].rearrange(
                        "(j p) f -> p j f", j=RG // 128),
                    dout[:])
            return
        # V_ext: [m-part, chunk, 64 V dims + ones col (+pad)] in bf16
        v_sb = qkvpool.tile([128, N_MC, 66], bf16, name="v_sb" + r)
        nc.vector.memset(v_sb[:, :, 64:65], 1.0)

        def mask_batch(g, b, eng=None):
            """Issue one batched mask DMA: chunks 4b..4b+3, rowgroup g."""
            mt4 = mpool.tile([128, MB, RG], bf16, name="mt4" + r, tag="mt")
            (eng or (nc.sync if (b % 2 == 0) else nc.gpsimd)).dma_start(
                mt4[:],
                MT[g, b, :, :].rearrange("(k p) q -> p k q", k=MB))
            return mt4

        # ---- Phase 1: K^T/Q^T and V from X^T ----
        mt_tiles = {}
        NT = N // NSPLIT
        with (
            tc.tile_pool(name="xt" + r, bufs=1) as xtpool,
            tc.tile_pool(name="qkvps" + r, bufs=4, space="PSUM") as qkvps,
        ):
            # column-blocked xt loads: QK tile t needs columns t*512 of
            # ALL four xt row-chunks, so land column block 0 (all c) first.
            xts = []
            for c in range(4):
                xts.append(xtpool.tile([128, N], bf16, name=f"xt_{c}" + r,
                                       tag=f"xt{c}"))
            blocks = [(0, 512), (512, 1024), (1024, 2048), (2048, 3072),
                      (3072, 4096)]
            for cb, (lo, hi) in enumerate(blocks):
                csl = slice(lo, hi)
                for c in range(4):
                    eng = nc.sync if ((cb * 4 + c) % 2 == 0) else nc.gpsimd
                    eng.dma_start(xts[c][:, csl],
                                  XT[c * 128:(c + 1) * 128, csl])
            if variant != "nomaskdma":
                for b in range(2):
                    mt_tiles[(0, b)] = mask_batch(0, b)
            else:
                mt4 = mpool.tile([128, MB, RG], bf16, name="mt4" + r,
                                 tag="mt", bufs=1)
                nc.vector.memset(mt4[:], 1.0)
                for g in range(N_RG):
                    for b in range(NB):
                        mt_tiles[(g, b)] = mt4
            for t in range(NT):
                ps = qkvps.tile([128, NSPLIT], f32, name="ps_qk" + r,
                                tag="qk")
                for c in range(4):
                    nc.tensor.matmul(
                        ps[:],
                        lhsT=wqk_sb[:, c, :],
                        rhs=xts[c][:, t * NSPLIT:(t + 1) * NSPLIT],
                        start=(c == 0), stop=(c == 3))
                sl = slice(t * NSPLIT, (t + 1) * NSPLIT)
                # ps rows 0:64 = K^T, rows 64:128 = Q^T (x8 into fp8 hi on
                # the phase-1-idle Act engine, residual into fp8 lo via
                # scratch on the DVE -- QKV PSUM tiles recycle 2x faster)
                nc.scalar.activation(kfall[0:64, 0, sl], ps[0:HD, :],
                                     AF.Copy, bias=0.0, scale=8.0)
                nc.vector.scalar_tensor_tensor(
                    fscr[0:HD, sl], ps[0:HD, :], 8.0,
                    kfall[0:64, 0, sl], op0=MUL, op1=SUB)
                nc.scalar.activation(qf2[64:128, 0, sl], ps[HD:128, :],
                                     AF.Copy, bias=0.0, scale=8.0)
                nc.vector.scalar_tensor_tensor(
                    fscr[HD:128, sl], ps[HD:128, :], 8.0,
                    qf2[64:128, 0, sl], op0=MUL, op1=SUB)
                # plane-dup / partition-shift DMAs in two column groups
                # (after tile 1 and after the last tile) so rowgroup 0's
                # E matmuls unlock early.
                if t == 1 or t == NT - 1:
                    dsl = (slice(0, 2 * NSPLIT) if t == 1
                           else slice(2 * NSPLIT, N))
                    nc.sync.dma_start(kfall[64:128, 0, dsl],
                                      kfall[0:64, 0, dsl])
                    nc.gpsimd.dma_start(kfall[64:128, 1, dsl],
                                        fscr[0:HD, dsl])
                    nc.sync.dma_start(qf2[0:64, 0, dsl],
                                      fscr[HD:128, dsl])
                    nc.gpsimd.dma_start(qf2[:, 1, dsl], qf2[:, 0, dsl])
                # V m4-group t uses the same xt column block as QK tile t
                psv = qkvps.tile([128, 4, HD], f32, name="ps_v" + r, tag="qk")
                for i in range(4):
                    m = t * 4 + i
                    for c in range(4):
                        nc.tensor.matmul(
                            psv[:, i, :],
                            lhsT=xts[c][:, m * 128:(m + 1) * 128],
                            rhs=wv_sb[:, c, :],
                            start=(c == 0), stop=(c == 3))
                nc.vector.tensor_copy(v_sb[:, t * 4:(t + 1) * 4, 0:HD],
                                      psv[:])

        # ---- Phase 2: attention main loop ----
        ctx2 = tc.tile_pool(name="eps" + r, bufs=3, space="PSUM")
        eps = ctx2.__enter__()
        ctx3 = tc.tile_pool(name="accps" + r, bufs=1, space="PSUM")
        accps = ctx3.__enter__()
        LAG = 5  # PE software-pipeline depth: PV_c emitted after E_{c+LAG};
        # 5 so the first PV of a new acc group sits far enough behind the E
        # stream that the in-order PE never blocks on the prior acc drain.
        NJ = RG // 128

        def finalize_steps(g, acc):
            """Per-rowgroup epilogue as a list of thunks; interleaved into
            the next rowgroup's chunk loop so the PE/Act pipeline never
            drains at rowgroup boundaries."""
            r0 = g * RG
            st = {}
            # last rowgroup's epilogue is the kernel tail: nothing overlaps
            # it, so split its work across DVE and Act and both DMA queues
            fast = (g == N_RG - 1)

            def s_copies():
                st["ot"] = fpool.tile([HD, RG], bf16, name="ot_sb" + r,
                                      tag="ot")
                st["s"] = fpool.tile([1, RG], f32, name="s_sb" + r, tag="s")
                if fast:
                    hl = RG // 2
                    nc.vector.tensor_copy(st["ot"][:, 0:hl], acc[0:HD, 0:hl])
                    nc.scalar.copy(st["ot"][:, hl:RG], acc[0:HD, hl:RG])
                    nc.vector.tensor_copy(st["s"][:, 0:hl],
                                          acc[HD:HD + 1, 0:hl])
                    nc.scalar.copy(st["s"][:, hl:RG], acc[HD:HD + 1, hl:RG])
                else:
                    nc.vector.tensor_copy(st["ot"][:], acc[0:HD, :])
                    nc.vector.tensor_copy(st["s"][:], acc[HD:HD + 1, :])

            def s_recip():
                stp = eps.tile([128, NJ], f32, name="st_ps" + r, tag="es")
                for j in range(NJ):
                    nc.tensor.transpose(
                        stp[:, j:j + 1],
                        st["s"][0:1, j * 128:(j + 1) * 128],
                        ident11[:])
                st["rt"] = fpool.tile([128, NJ], f32, name="rt_sb" + r,
                                      tag="rt")
                nc.vector.reciprocal(st["rt"][:], stp[:])
                st["out"] = opool.tile([128, NJ, F_OUT], bf16,
                                       name="out_t" + r, tag="out")

            def s_wo(j):
                def f():
                    pso = eps.tile([128, F_OUT], f32, name="pso" + r,
                                   tag="es")
                    nc.tensor.matmul(
                        pso[:],
                        lhsT=st["ot"][:, j * 128:(j + 1) * 128],
                        rhs=wo_sb[:],
                        start=True, stop=True)
                    # alternate the pso drain+scale between DVE and Act:
                    # the DVE mask multiplies are the steady-state wall,
                    # and Act has slack after its (HW-cheap) exps
                    if j % 2 == 1:
                        nc.scalar.activation(
                            st["out"][:, j, :], pso[:], AF.Copy,
                            bias=0.0, scale=st["rt"][:, j:j + 1])
                    else:
                        nc.vector.tensor_scalar_mul(
                            st["out"][:, j, :], pso[:],
                            st["rt"][:, j:j + 1])
                return f

            def s_store():
                if fast:
                    h = NJ // 2
                    nc.sync.dma_start(
                        OUT[r0:r0 + RG // 2, :].rearrange(
                            "(j p) f -> p j f", j=h),
                        st["out"][:, 0:h, :])
                    nc.gpsimd.dma_start(
                        OUT[r0 + RG // 2:r0 + RG, :].rearrange(
                            "(j p) f -> p j f", j=h),
                        st["out"][:, h:NJ, :])
                else:
                    nc.gpsimd.dma_start(
                        OUT[r0:r0 + RG, :].rearrange(
                            "(j p) f -> p j f", j=NJ),
                        st["out"][:])

            return ([s_copies, s_recip]
                    + [s_wo(j) for j in range(NJ)] + [s_store])

        # Flat pipeline over all (g, c) chunks: E/exp/mask lead, PV lags
        # LAG chunks behind (crossing rowgroup boundaries), the previous
        # rowgroup's finalize interleaves one step per slot.
        NTOT = N_RG * N_MC
        accs = {}
        pts = {}
        pending = []
        for k in range(NTOT + LAG):
            if k < NTOT:
                g, c = divmod(k, N_MC)
                r0 = g * RG
                b = c // MB
                if variant != "nomaskdma":
                    # prefetch ~3 batches ahead
                    nxt = (g, b + 3) if b + 3 < NB else (g + 1, b + 3 - NB)
                    if c % MB == 0 and nxt[0] < N_RG and nxt not in mt_tiles:
                        mt_tiles[nxt] = mask_batch(*nxt)
                    if (g, b) not in mt_tiles:
                        mt_tiles[(g, b)] = mask_batch(g, b)
                mt4 = mt_tiles[(g, b)]
                es = eps.tile([128, RG], f32, name="es" + r, tag="es")
                for s in range(RG // NSPLIT):
                    ssl = slice(s * NSPLIT, (s + 1) * NSPLIT)
                    for _dup in range(2 if variant == "e2x" else 1):
                        nc.tensor.matmul(
                            es[:, ssl],
                            lhsT=kfall[:, :, c * 128:(c + 1) * 128],
                            rhs=qf2[:, :, r0 + s * NSPLIT:
                                    r0 + (s + 1) * NSPLIT],
                            start=True, stop=True,
                            perf_mode=DR, skip_group_check=True)
                if variant == "eonly":
                    continue
                p_t = ppool.tile([128, RG], bf16, name="p_t" + r, tag="p")
                nc.scalar.activation(p_t[:], es[:], AF.Exp,
                                     bias=zeros_p[:], scale=SCALE / 64.0)
                if variant == "exp2x":
                    nc.scalar.activation(p_t[:], es[:], AF.Exp,
                                         bias=zeros_p[:], scale=SCALE / 64.0)
                nc.vector.tensor_mul(p_t[:], p_t[:], mt4[:, c % MB, :])
                if variant == "mul2x":
                    nc.vector.tensor_mul(p_t[:], p_t[:], mt4[:, c % MB, :])
                pts[k] = p_t
            if k >= LAG and variant != "eonly":
                kv = k - LAG
                gv, cv = divmod(kv, N_MC)
                if cv == 0:
                    accs[gv] = accps.tile([HD + 1, RG], f32, name="acc" + r,
                                          tag="acc")
                p_t = pts.pop(kv)
                for s in range(RG // NSPLIT):
                    nc.tensor.matmul(
                        accs[gv][:, s * NSPLIT:(s + 1) * NSPLIT],
                        lhsT=v_sb[:, cv, 0:HD + 1],
                        rhs=p_t[:, s * NSPLIT:(s + 1) * NSPLIT],
                        start=(cv == 0), stop=(cv == N_MC - 1),
                        skip_group_check=True)
                if cv == N_MC - 1:
                    for f in pending:   # rare leftover (shouldn't happen)
                        f()
                    pending = finalize_steps(gv, accs.pop(gv))
            if pending:
                pending.pop(0)()
        for f in pending:
            f()
        ctx3.__exit__(None, None, None)
        ctx2.__exit__(None, None, None)


def _shard_inputs(X, mask, W_Q, W_K, W_V, W_O):
    """Per-core input dicts (host-side layout prep)."""
    in_maps = []
    for h in range(H):
        xt = X[h].T.astype(BF16)                               # [512, 4096]
        # mask[h].T as bf16 bits: 1 -> 0x3F80 (bf16 1.0), 0 -> 0
        m16 = mask[h].view(np.uint16)[:, 0::2]                 # low half of i32
        mt = (m16.T * np.uint16(0x3F80)).view(BF16)            # [4096, 4096]
        # tile: [keys, queries] -> [g, b, 512, 1024] contiguous per batch
        mt = np.ascontiguousarray(
            mt.reshape(NB, MB * 128, N_RG, RG).transpose(2, 0, 1, 3))
        wq = W_Q[h].T.reshape(4, 128, HD).transpose(1, 0, 2)   # [128, 4, 64]
        wk = W_K[h].T.reshape(4, 128, HD).transpose(1, 0, 2)
        wqk = np.concatenate([wk, wq], axis=2).astype(BF16)    # [128, 4, 128]
        wv = np.ascontiguousarray(
            W_V[h].T.reshape(4, 128, HD).transpose(1, 0, 2)).astype(BF16)
        wo = W_O[:, h * HD:(h + 1) * HD].T.astype(BF16)        # [64, 512]
        in_maps.append({"xt": xt, "mt": mt, "wqk": wqk,
                        "wv": wv, "wo": wo})
    return in_maps


def kernel(X, mask, W_Q, W_K, W_V, W_O):
    from concourse.bass_utils import run_bass_kernel_spmd
    nc = _build_program(repeat=1)
    in_maps = _shard_inputs(X, mask, W_Q, W_K, W_V, W_O)
    res = run_bass_kernel_spmd(nc, in_maps, list(range(N_CORES)))
    out = np.zeros((N, F_OUT), np.float64)
    for h in range(H):
        out += res.results[h]["out"].astype(np.float64)
    return out.astype(np.float32)


# revision 39
# speedup vs baseline: 1.0054x; 1.0054x over previous
"""Multi-head sparse attention on 8 NeuronCores (Trainium2, Bass/Tile).

Head-parallel sharding: core h owns head h (H == n_cores == 8).
Each core computes its head's attention output and the partial final
projection through its W_O column slice; the host sums the 8 partials.

Math note: softmax rows are never fully masked (random 0/1 mask) and
E = QK^T/8 is small, so softmax is computed WITHOUT max subtraction:
P = exp(E) * mask, out = (P @ V) / rowsum(P).  rowsum comes from a
ones-column appended to V, and the normalization is deferred until
after the W_O projection (a per-row scalar).

v7: measured-HW layout.  PE matmul cost is out_free x 1 cycle
regardless of contraction depth or DoubleRow, so E runs as a plain
bf16 matmul with contraction 64 (no fp8 hi/lo folding, no plane-dup
DMAs) -- same speed as the fp8 fold, better precision, far less
phase-1 work.  The mask rides the DVE as a bf16 multiply (2x mode);
the Act engine (exp is only ~640ns/chunk on HW) also absorbs the QK
PSUM drains and the 32 output scalings.  Scheduling: LAG=5 so the
in-order PE never head-of-line blocks on acc drains, V matmuls
interleave into the QK tile loop, xt loads are column-blocked so the
first QK tile lands early, and the last rowgroup's epilogue is split
across DVE/Act/both DMA queues (nothing overlaps the tail).
"""

import numpy as np
import ml_dtypes

H, N, F_IN, HD, F_OUT = 8, 4096, 512, 64, 512
N_CORES = 8
RG = 1024            # query-row group processed per PSUM accumulator
N_RG = N // RG       # 4
MC = 128             # key/m chunk (partition dim)
N_MC = N // MC       # 32
NSPLIT = 512         # matmul moving-operand free size
MB = 4               # mask chunks per DMA batch
NB = N_MC // MB      # mask batches per rowgroup (8)
BF16 = ml_dtypes.bfloat16

_PROGRAM_CACHE = {}


def _build_program(repeat=1, timing=False, variant="full"):
    """Build + compile the Bass/Tile program (same SPMD program for all cores).

    timing=True builds a benchmark variant: inputs live in internal DRAM
    (initialized on-device), the body runs `repeat` times inside a hardware
    For_i loop, and only a tiny checksum output is external.  Differencing
    the wall time of two repeat counts isolates the per-iteration HW time.
    """
    key = (repeat, timing, variant)
    if key in _PROGRAM_CACHE:
        return _PROGRAM_CACHE[key]

    import concourse.bacc as bacc
    import concourse.tile as tile
    import concourse.mybir as mybir

    f32 = mybir.dt.float32
    bf16 = mybir.dt.bfloat16

    nc = bacc.Bacc("TRN2", target_bir_lowering=False, debug=False,
                   num_devices=N_CORES)

    kind_in = {} if timing else {"kind": "ExternalInput"}
    XT = nc.dram_tensor("xt", [F_IN, N], bf16, **kind_in).ap()
    MT = nc.dram_tensor("mt", [N_RG, NB, MB * 128, RG], bf16,
                        **kind_in).ap()
    WQK = nc.dram_tensor("wqk", [128, 4, 128], bf16, **kind_in).ap()
    WV = nc.dram_tensor("wv", [128, 4, HD], bf16, **kind_in).ap()
    WO = nc.dram_tensor("wo", [HD, F_OUT], bf16, **kind_in).ap()
    if not timing:
        OUT = nc.dram_tensor("out", [N, F_OUT], bf16,
                             kind="ExternalOutput").ap()
    else:
        OUT = nc.dram_tensor("out", [N, F_OUT], bf16).ap()
        DUMMY = nc.dram_tensor("dumin", [1, 8], f32, kind="ExternalInput").ap()
        CHK = nc.dram_tensor("chk", [128, F_OUT], bf16,
                             kind="ExternalOutput").ap()

    SCALE = float(1.0 / np.sqrt(HD))

    with tile.TileContext(nc) as tc:
        with (
            tc.tile_pool(name="consts", bufs=1) as consts,
            tc.tile_pool(name="wpool", bufs=1) as wpool,
        ):
            ident11 = consts.tile([1, 1], f32)
            nc.vector.memset(ident11[:], 1.0)
            zeros_p = consts.tile([128, 1], f32)
            nc.vector.memset(zeros_p[:], 0.0)

            wqk_sb = wpool.tile([128, 4, 128], bf16)
            wv_sb = wpool.tile([128, 4, HD], bf16)
            wo_sb = wpool.tile([HD, F_OUT], bf16)
            # fp8 hi/lo folded operands for the DoubleRow E matmul:
            # kfall: stationary planes [128, j, N]; rows 0:64 = (Khi, 0),
            #        rows 64:128 = (Khi copy, Klo)
            # qf2:   moving planes; rows 0:64 = Qlo, rows 64:128 = Qhi,
            #        plane j1 = copy of j0
            fp8 = mybir.dt.float8e4
            kfall = wpool.tile([128, 2, N], fp8)
            qf2 = wpool.tile([128, 2, N], fp8)
            fscr = wpool.tile([128, N], fp8)
            nc.vector.memset(kfall[0:64, 1, :], 0.0)

            if timing:
                # on-device init of internal DRAM inputs (runs once)
                with tc.tile_pool(name="init", bufs=1) as initp:
                    mrow = initp.tile([128, N], bf16)
                    nc.vector.memset(mrow[:], 1.0)
                    MTf = MT.rearrange("g b p q -> (g b p) q")
                    for c in range(N * N // (128 * RG)):
                        nc.sync.dma_start(MTf[c * 128:(c + 1) * 128, :],
                                          mrow[:, 0:RG])
                    xrow = initp.tile([128, N], bf16)
                    nc.vector.memset(xrow[:], 0.015625)
                    for c in range(4):
                        nc.sync.dma_start(XT[c * 128:(c + 1) * 128, :], xrow[:])
                    wrow = initp.tile([128, 4 * 128], bf16)
                    nc.vector.memset(wrow[:], 0.03125)
                    nc.sync.dma_start(WQK.rearrange("p c d -> p (c d)"),
                                      wrow[:])
                    nc.sync.dma_start(WV.rearrange("p c d -> p (c d)"),
                                      wrow[:, 0:4 * HD])
                    worow = initp.tile([HD, F_OUT], bf16)
                    nc.vector.memset(worow[:], 0.03125)
                    nc.sync.dma_start(WO[:], worow[:])

            nc.sync.dma_start(wqk_sb[:], WQK[:])
            nc.sync.dma_start(wv_sb[:], WV[:])
            nc.sync.dma_start(wo_sb[:], WO[:])

            if timing and repeat > 1:
                with tc.For_i(0, repeat, 1):
                    _one_pass(nc, tc, mybir, XT, MT, OUT,
                              wqk_sb, wv_sb, wo_sb, kfall, qf2, fscr,
                              ident11, zeros_p, SCALE, 0, variant)
            else:
                for rep in range(repeat):
                    _one_pass(nc, tc, mybir, XT, MT, OUT,
                              wqk_sb, wv_sb, wo_sb, kfall, qf2, fscr,
                              ident11, zeros_p, SCALE, rep, variant)

            if timing:
                with tc.tile_pool(name="chkp", bufs=1) as chkp:
                    chk_sb = chkp.tile([128, F_OUT], bf16)
                    nc.sync.dma_start(chk_sb[:], OUT[0:128, :])
                    nc.sync.dma_start(CHK[:], chk_sb[:])

    nc.compile()
    _PROGRAM_CACHE[key] = nc
    return nc


def _one_pass(nc, tc, mybir, XT, MT, OUT,
              wqk_sb, wv_sb, wo_sb, kfall, qf2, fscr,
              ident11, zeros_p, SCALE, rep,
              variant="full"):
    f32 = mybir.dt.float32
    bf16 = mybir.dt.bfloat16
    AF = mybir.ActivationFunctionType
    MUL = mybir.AluOpType.mult
    SUB = mybir.AluOpType.subtract
    DR = mybir.MatmulPerfMode.DoubleRow
    r = f"_r{rep}"

    with (
        tc.tile_pool(name="qkv" + r, bufs=1) as qkvpool,
        tc.tile_pool(name="mpool" + r, bufs=4) as mpool,
        tc.tile_pool(name="ppool" + r, bufs=7) as ppool,
        tc.tile_pool(name="fpool" + r, bufs=2) as fpool,
        tc.tile_pool(name="opool" + r, bufs=2) as opool,
    ):
        if variant == "dmaonly":
            dout = opool.tile([128, RG // 128, F_OUT], bf16,
                              name="dout" + r, tag="out")
            nc.vector.memset(dout[:], 0.0)
            for c in range(4):
                xt_c = fpool.tile([128, N], bf16, name=f"dxt_{c}" + r,
                                  tag="tmp")
                eng = nc.sync if (c % 2 == 0) else nc.gpsimd
                eng.dma_start(xt_c[:], XT[c * 128:(c + 1) * 128, :])
            for g in range(N_RG):
                for b in range(NB):
                    mt4 = mpool.tile([128, MB, RG], bf16, name="mt4" + r,
                                     tag="mt")
                    eng = nc.sync if (b % 2 == 0) else nc.gpsimd
                    eng.dma_start(
                        mt4[:],
                        MT[g, b, :, :].rearrange("(k p) q -> p k q", k=MB))
                nc.gpsimd.dma_start(
                    OUT[g * RG:(g + 1) * RG, :].rearrange(
                        "(j p) f -> p j f", j=RG // 128),
                    dout[:])
            return
        # V_ext: [m-part, chunk, 64 V dims + ones col (+pad)] in bf16
        v_sb = qkvpool.tile([128, N_MC, 66], bf16, name="v_sb" + r)
        nc.vector.memset(v_sb[:, :, 64:65], 1.0)

        def mask_batch(g, b, eng=None):
            """Issue one batched mask DMA: chunks 4b..4b+3, rowgroup g."""
            mt4 = mpool.tile([128, MB, RG], bf16, name="mt4" + r, tag="mt")
            (eng or (nc.sync if (b % 2 == 0) else nc.gpsimd)).dma_start(
                mt4[:],
                MT[g, b, :, :].rearrange("(k p) q -> p k q", k=MB))
            return mt4

        # ---- Phase 1: K^T/Q^T and V from X^T ----
        mt_tiles = {}
        NT = N // NSPLIT
        with (
            tc.tile_pool(name="xt" + r, bufs=1) as xtpool,
            tc.tile_pool(name="qkvps" + r, bufs=4, space="PSUM") as qkvps,
        ):
            # column-blocked xt loads: QK tile t needs columns t*512 of
            # ALL four xt row-chunks, so land column block 0 (all c) first.
            xts = []
            for c in range(4):
                xts.append(xtpool.tile([128, N], bf16, name=f"xt_{c}" + r,
                                       tag=f"xt{c}"))
            blocks = [(0, 512), (512, 1024), (1024, 2048), (2048, 3072),
                      (3072, 4096)]
            for cb, (lo, hi) in enumerate(blocks):
                csl = slice(lo, hi)
                for c in range(4):
                    eng = nc.sync if ((cb * 4 + c) % 2 == 0) else nc.gpsimd
                    eng.dma_start(xts[c][:, csl],
                                  XT[c * 128:(c + 1) * 128, csl])
            if variant != "nomaskdma":
                for b in range(2):
                    mt_tiles[(0, b)] = mask_batch(0, b)
            else:
                mt4 = mpool.tile([128, MB, RG], bf16, name="mt4" + r,
                                 tag="mt", bufs=1)
                nc.vector.memset(mt4[:], 1.0)
                for g in range(N_RG):
                    for b in range(NB):
                        mt_tiles[(g, b)] = mt4
            for t in range(NT):
                ps = qkvps.tile([128, NSPLIT], f32, name="ps_qk" + r,
                                tag="qk")
                for c in range(4):
                    nc.tensor.matmul(
                        ps[:],
                        lhsT=wqk_sb[:, c, :],
                        rhs=xts[c][:, t * NSPLIT:(t + 1) * NSPLIT],
                        start=(c == 0), stop=(c == 3))
                sl = slice(t * NSPLIT, (t + 1) * NSPLIT)
                # ps rows 0:64 = K^T, rows 64:128 = Q^T (x8 into fp8 hi on
                # the phase-1-idle Act engine, residual into fp8 lo via
                # scratch on the DVE -- QKV PSUM tiles recycle 2x faster)
                nc.scalar.activation(kfall[0:64, 0, sl], ps[0:HD, :],
                                     AF.Copy, bias=0.0, scale=8.0)
                nc.vector.scalar_tensor_tensor(
                    fscr[0:HD, sl], ps[0:HD, :], 8.0,
                    kfall[0:64, 0, sl], op0=MUL, op1=SUB)
                nc.scalar.activation(qf2[64:128, 0, sl], ps[HD:128, :],
                                     AF.Copy, bias=0.0, scale=8.0)
                nc.vector.scalar_tensor_tensor(
                    fscr[HD:128, sl], ps[HD:128, :], 8.0,
                    qf2[64:128, 0, sl], op0=MUL, op1=SUB)
                # plane-dup / partition-shift DMAs in two column groups
                # (after tile 1 and after the last tile) so rowgroup 0's
                # E matmuls unlock early.
                if t == 1 or t == NT - 1:
                    dsl = (slice(0, 2 * NSPLIT) if t == 1
                           else slice(2 * NSPLIT, N))
                    nc.sync.dma_start(kfall[64:128, 0, dsl],
                                      kfall[0:64, 0, dsl])
                    nc.gpsimd.dma_start(kfall[64:128, 1, dsl],
                                        fscr[0:HD, dsl])
                    nc.sync.dma_start(qf2[0:64, 0, dsl],
                                      fscr[HD:128, dsl])
                    nc.gpsimd.dma_start(qf2[:, 1, dsl], qf2[:, 0, dsl])
                # V m4-group t uses the same xt column block as QK tile t
                psv = qkvps.tile([128, 4, HD], f32, name="ps_v" + r, tag="qk")
                for i in range(4):
                    m = t * 4 + i
                    for c in range(4):
                        nc.tensor.matmul(
                            psv[:, i, :],
                            lhsT=xts[c][:, m * 128:(m + 1) * 128],
                            rhs=wv_sb[:, c, :],
                            start=(c == 0), stop=(c == 3))
                nc.vector.tensor_copy(v_sb[:, t * 4:(t + 1) * 4, 0:HD],
                                      psv[:])

        # ---- Phase 2: attention main loop ----
        ctx2 = tc.tile_pool(name="eps" + r, bufs=3, space="PSUM")
        eps = ctx2.__enter__()
        ctx3 = tc.tile_pool(name="accps" + r, bufs=1, space="PSUM")
        accps = ctx3.__enter__()
        LAG = 5  # PE software-pipeline depth: PV_c emitted after E_{c+LAG};
        # 5 so the first PV of a new acc group sits far enough behind the E
        # stream that the in-order PE never blocks on the prior acc drain.
        NJ = RG // 128

        def finalize_steps(g, acc):
            """Per-rowgroup epilogue as a list of thunks; interleaved into
            the next rowgroup's chunk loop so the PE/Act pipeline never
            drains at rowgroup boundaries."""
            r0 = g * RG
            st = {}
            # last rowgroup's epilogue is the kernel tail: nothing overlaps
            # it, so split its work across DVE and Act and both DMA queues
            fast = (g == N_RG - 1)

            def s_copies():
                st["ot"] = fpool.tile([HD, RG], bf16, name="ot_sb" + r,
                                      tag="ot")
                st["s"] = fpool.tile([1, RG], f32, name="s_sb" + r, tag="s")
                if fast:
                    hl = RG // 2
                    nc.vector.tensor_copy(st["ot"][:, 0:hl], acc[0:HD, 0:hl])
                    nc.scalar.copy(st["ot"][:, hl:RG], acc[0:HD, hl:RG])
                    nc.vector.tensor_copy(st["s"][:, 0:hl],
                                          acc[HD:HD + 1, 0:hl])
                    nc.scalar.copy(st["s"][:, hl:RG], acc[HD:HD + 1, hl:RG])
                else:
                    nc.vector.tensor_copy(st["ot"][:], acc[0:HD, :])
                    nc.vector.tensor_copy(st["s"][:], acc[HD:HD + 1, :])

            def s_recip():
                stp = eps.tile([128, NJ], f32, name="st_ps" + r, tag="es")
                for j in range(NJ):
                    nc.tensor.transpose(
                        stp[:, j:j + 1],
                        st["s"][0:1, j * 128:(j + 1) * 128],
                        ident11[:])
                st["rt"] = fpool.tile([128, NJ], f32, name="rt_sb" + r,
                                      tag="rt")
                nc.vector.reciprocal(st["rt"][:], stp[:])
                st["out"] = opool.tile([128, NJ, F_OUT], bf16,
                                       name="out_t" + r, tag="out")

            def s_wo(j):
                def f():
                    pso = eps.tile([128, F_OUT], f32, name="pso" + r,
                                   tag="es")
                    nc.tensor.matmul(
                        pso[:],
                        lhsT=st["ot"][:, j * 128:(j + 1) * 128],
                        rhs=wo_sb[:],
                        start=True, stop=True)
                    # keep the steady-state drain+scale on DVE (an Act
                    # Copy here head-of-line-blocks the exp stream); only
                    # the un-overlapped tail splits odd j's onto Act
                    if fast and j % 2 == 1:
                        nc.scalar.activation(
                            st["out"][:, j, :], pso[:], AF.Copy,
                            bias=0.0, scale=st["rt"][:, j:j + 1])
                    else:
                        nc.vector.tensor_scalar_mul(
                            st["out"][:, j, :], pso[:],
                            st["rt"][:, j:j + 1])
                return f

            def s_store():
                if fast:
                    h = NJ // 2
                    nc.sync.dma_start(
                        OUT[r0:r0 + RG // 2, :].rearrange(
                            "(j p) f -> p j f", j=h),
                        st["out"][:, 0:h, :])
                    nc.gpsimd.dma_start(
                        OUT[r0 + RG // 2:r0 + RG, :].rearrange(
                            "(j p) f -> p j f", j=h),
                        st["out"][:, h:NJ, :])
                else:
                    nc.gpsimd.dma_start(
                        OUT[r0:r0 + RG, :].rearrange(
                            "(j p) f -> p j f", j=NJ),
                        st["out"][:])

            return ([s_copies, s_recip]
                    + [s_wo(j) for j in range(NJ)] + [s_store])

        # Flat pipeline over all (g, c) chunks: E/exp/mask lead, PV lags
        # LAG chunks behind (crossing rowgroup boundaries), the previous
        # rowgroup's finalize interleaves one step per slot.
        NTOT = N_RG * N_MC
        accs = {}
        pts = {}
        pending = []
        for k in range(NTOT + LAG):
            if k < NTOT:
                g, c = divmod(k, N_MC)
                r0 = g * RG
                b = c // MB
                if variant != "nomaskdma":
                    # prefetch ~3 batches ahead
                    nxt = (g, b + 3) if b + 3 < NB else (g + 1, b + 3 - NB)
                    if c % MB == 0 and nxt[0] < N_RG and nxt not in mt_tiles:
                        mt_tiles[nxt] = mask_batch(*nxt)
                    if (g, b) not in mt_tiles:
                        mt_tiles[(g, b)] = mask_batch(g, b)
                mt4 = mt_tiles[(g, b)]
                es = eps.tile([128, RG], f32, name="es" + r, tag="es")
                for s in range(RG // NSPLIT):
                    ssl = slice(s * NSPLIT, (s + 1) * NSPLIT)
                    for _dup in range(2 if variant == "e2x" else 1):
                        nc.tensor.matmul(
                            es[:, ssl],
                            lhsT=kfall[:, :, c * 128:(c + 1) * 128],
                            rhs=qf2[:, :, r0 + s * NSPLIT:
                                    r0 + (s + 1) * NSPLIT],
                            start=True, stop=True,
                            perf_mode=DR, skip_group_check=True)
                if variant == "eonly":
                    continue
                p_t = ppool.tile([128, RG], bf16, name="p_t" + r, tag="p")
                nc.scalar.activation(p_t[:], es[:], AF.Exp,
                                     bias=zeros_p[:], scale=SCALE / 64.0)
                if variant == "exp2x":
                    nc.scalar.activation(p_t[:], es[:], AF.Exp,
                                         bias=zeros_p[:], scale=SCALE / 64.0)
                nc.vector.tensor_mul(p_t[:], p_t[:], mt4[:, c % MB, :])
                if variant == "mul2x":
                    nc.vector.tensor_mul(p_t[:], p_t[:], mt4[:, c % MB, :])
                pts[k] = p_t
            if k >= LAG and variant != "eonly":
                kv = k - LAG
                gv, cv = divmod(kv, N_MC)
                if cv == 0:
                    accs[gv] = accps.tile([HD + 1, RG], f32, name="acc" + r,
                                          tag="acc")
                p_t = pts.pop(kv)
                for s in range(RG // NSPLIT):
                    nc.tensor.matmul(
                        accs[gv][:, s * NSPLIT:(s + 1) * NSPLIT],
                        lhsT=v_sb[:, cv, 0:HD + 1],
                        rhs=p_t[:, s * NSPLIT:(s + 1) * NSPLIT],
                        start=(cv == 0), stop=(cv == N_MC - 1),
                        skip_group_check=True)
                if cv == N_MC - 1:
                    for f in pending:   # rare leftover (shouldn't happen)
                        f()
                    pending = finalize_steps(gv, accs.pop(gv))
            if pending:
                pending.pop(0)()
        for f in pending:
            f()
        ctx3.__exit__(None, None, None)
        ctx2.__exit__(None, None, None)


def _shard_inputs(X, mask, W_Q, W_K, W_V, W_O):
    """Per-core input dicts (host-side layout prep)."""
    in_maps = []
    for h in range(H):
        xt = X[h].T.astype(BF16)                               # [512, 4096]
        # mask[h].T as bf16 bits: 1 -> 0x3F80 (bf16 1.0), 0 -> 0
        m16 = mask[h].view(np.uint16)[:, 0::2]                 # low half of i32
        mt = (m16.T * np.uint16(0x3F80)).view(BF16)            # [4096, 4096]
        # tile: [keys, queries] -> [g, b, 512, 1024] contiguous per batch
        mt = np.ascontiguousarray(
            mt.reshape(NB, MB * 128, N_RG, RG).transpose(2, 0, 1, 3))
        wq = W_Q[h].T.reshape(4, 128, HD).transpose(1, 0, 2)   # [128, 4, 64]
        wk = W_K[h].T.reshape(4, 128, HD).transpose(1, 0, 2)
        wqk = np.concatenate([wk, wq], axis=2).astype(BF16)    # [128, 4, 128]
        wv = np.ascontiguousarray(
            W_V[h].T.reshape(4, 128, HD).transpose(1, 0, 2)).astype(BF16)
        wo = W_O[:, h * HD:(h + 1) * HD].T.astype(BF16)        # [64, 512]
        in_maps.append({"xt": xt, "mt": mt, "wqk": wqk,
                        "wv": wv, "wo": wo})
    return in_maps


def kernel(X, mask, W_Q, W_K, W_V, W_O):
    from concourse.bass_utils import run_bass_kernel_spmd
    nc = _build_program(repeat=1)
    in_maps = _shard_inputs(X, mask, W_Q, W_K, W_V, W_O)
    res = run_bass_kernel_spmd(nc, in_maps, list(range(N_CORES)))
    out = np.zeros((N, F_OUT), np.float64)
    for h in range(H):
        out += res.results[h]["out"].astype(np.float64)
    return out.astype(np.float32)
